# revision 38
# baseline (speedup 1.0000x reference)
"""Trainium2 Bass kernel for DGCF (nn_DGCF_44830868636068).

Algorithm (per reference.py): 2 layers x 2 routing iterations of factor-wise
graph message passing over a 100K-node / 2M-edge graph, EMB_DIM=128 split
into 4 factors of 32.

Distribution: nodes sharded across 8 cores by head ownership (12544
nodes/core = 98 windows of 128). Edges live on the core owning their head h.
Per core, edges are grouped by (h-window, t-chunk) where the 100352-node
table is split into 4 chunks of 25088 rows so tail indices fit int16 for
dma_gather. Every (window, chunk) segment is padded to T_cw tiles of 128
slots (global max, identical on all cores -> one SPMD program).

Per iteration:
  scores = softmax(A) per edge over 4 factors
  deg pass: one-hot(h) matmuls accumulate deg[node,4] (PE)
  d_inv = 1/sqrt(max(deg,eps)); y = ego * d_inv (bf16) -> AllGather y
  message pass: per (window, chunk), dma_gather G = y_chunk[t_local]
    (256B bf16 rows, int16 idxs, pads gather row 0), rhs = G * score,
    S_win = sum_{c,k} M^T @ rhs (PSUM f32)
    (it0 of layer: tau = tanh(l2norm_chunk(G)) spilled to DRAM)
  node phase: s_hat = S/||S||_chunk (bf16); on layer-final its x_new = S*d_inv
  Phase B (skipped on very last it): head = M @ s_hat_win via DMA-transposed
    one-hot, new_val = chunk_dot(head, tau), A += new_val

Outputs ego after layer1 and layer2; host computes (ego0+ego1+ego2)/3.
"""

import math
import numpy as np
import ml_dtypes

from concourse import bass, mybir, bacc
from concourse import tile
from concourse.bass_utils import run_bass_kernel_spmd

P = 128
D = 128
F = 4
DF = 32
EPS = 1e-12
f32 = mybir.dt.float32
bf16 = mybir.dt.bfloat16
i16 = mybir.dt.int16

N_USERS = 50000
N_ITEMS = 50000
N_REAL = N_USERS + N_ITEMS
N_LAYERS = 2
N_ITERS = 2


class Cfg:
    def __init__(self, cores, nw, t_cw, nch=4, n_layers=N_LAYERS, n_iters=N_ITERS):
        self.cores = cores
        self.nw = nw            # windows per core
        self.nch = nch          # t-chunks (each <= 32768 rows)
        self.t_cw = t_cw        # tiles per (window, chunk) segment
        self.tw = t_cw * nch    # tiles per window
        self.ns = nw * P        # nodes per core
        self.n_pad = self.ns * cores
        self.chunk = self.n_pad // nch
        assert self.chunk <= 32768
        self.ntiles = nw * self.tw
        self.e_pad = self.ntiles * P
        self.seg = t_cw * P     # slots per (w, c) segment
        self.n_layers = n_layers
        self.n_iters = n_iters


def cfg_from_key(key):
    return Cfg(key[0], key[1], key[2])


def input_specs(cfg: Cfg):
    S16 = cfg.seg // 16
    return [
        ("ego0", [cfg.ns, D], f32),
        ("tidx", [P, cfg.nw * cfg.nch * S16], i16),
        ("hmod", [P, cfg.ntiles], bf16),
        ("iota", [P, P], bf16),
        ("ident", [P, P], bf16),
    ]


def output_specs(cfg: Cfg):
    return [(f"out{l}", [cfg.ns, D], f32) for l in range(cfg.n_layers)]


# ---------------------------------------------------------------- host-side


def compute_t_cw(h, t, cfg_proto):
    h = np.asarray(h).astype(np.int64)
    t = np.asarray(t).astype(np.int64)
    core_of = h // cfg_proto.ns
    maxc = 0
    for c in range(cfg_proto.cores):
        m = core_of == c
        win = (h[m] - c * cfg_proto.ns) // P
        chunk = t[m] // cfg_proto.chunk
        cnt = np.zeros((cfg_proto.nw, cfg_proto.nch), np.int64)
        np.add.at(cnt, (win, chunk), 1)
        maxc = max(maxc, int(cnt.max()))
    return math.ceil(maxc / P)


def preprocess(h, t, cfg: Cfg):
    """Per-core edge arrays in device layout.

    Slot (w, c, k, p) holds edge j = ((w*nch + c)*t_cw + k)*P + p.
    Returns per core:
      hmod  [P, ntiles] bf16  (pad 255)
      tidx  [P, nw*nch*S16] int16 wrapped+replicated layout for dma_gather,
            S16 = seg//16 columns per (w,c) segment (pad idx 0)
    """
    h = np.asarray(h).astype(np.int64)
    t = np.asarray(t).astype(np.int64)
    core_of = h // cfg.ns
    S16 = cfg.seg // 16
    out = []
    for c in range(cfg.cores):
        m = core_of == c
        hc, tc = h[m], t[m]
        win = (hc - c * cfg.ns) // P
        hmod = (hc - c * cfg.ns) % P
        chunk = tc // cfg.chunk
        tloc = tc % cfg.chunk
        order = np.lexsort((chunk, win))
        win, hmod, chunk, tloc = win[order], hmod[order], chunk[order], tloc[order]
        cnt = np.zeros((cfg.nw, cfg.nch), np.int64)
        np.add.at(cnt, (win, chunk), 1)
        assert cnt.max() <= cfg.seg

        hmod_pad = np.full(cfg.e_pad, 255, dtype=np.int64)
        tloc_pad = np.zeros(cfg.e_pad, dtype=np.int64)
        pos = 0
        for w in range(cfg.nw):
            for ch in range(cfg.nch):
                k = int(cnt[w, ch])
                base = (w * cfg.nch + ch) * cfg.seg
                hmod_pad[base : base + k] = hmod[pos : pos + k]
                tloc_pad[base : base + k] = tloc[pos : pos + k]
                pos += k
        assert pos == len(win)

        hm = (
            hmod_pad.reshape(cfg.ntiles, P).T.astype(ml_dtypes.bfloat16).copy()
        )  # [P, ntiles]
        # wrapped idx layout per (w,c) segment: slot i -> [i%16, i//16],
        # replicated to all 8 groups of 16 partitions
        tl = tloc_pad.reshape(cfg.nw * cfg.nch, cfg.seg)  # [segments, seg]
        wrapped = np.zeros((P, cfg.nw * cfg.nch * S16), np.int16)
        for s in range(cfg.nw * cfg.nch):
            blk = tl[s].reshape(S16, 16).T.astype(np.int16)  # [16, S16]
            wrapped[:, s * S16 : (s + 1) * S16] = np.tile(blk, (8, 1))
        out.append({"hmod": hm, "tidx": wrapped})
    return out


def shard_ego(user_emb, item_emb, cfg: Cfg):
    ego = np.concatenate([np.asarray(user_emb), np.asarray(item_emb)], 0).astype(
        np.float32
    )
    n_real = ego.shape[0]
    ego = np.concatenate([ego, np.zeros((cfg.n_pad - n_real, D), np.float32)], 0)
    return [ego[c * cfg.ns : (c + 1) * cfg.ns].copy() for c in range(cfg.cores)]


# ---------------------------------------------------------------- device


def build_program(cfg: Cfg, debug_dump=False, ablate=()):
    nc = bacc.Bacc(
        "TRN2",
        target_bir_lowering=False,
        debug=False,
        num_devices=cfg.cores,
        num_swdge_queues=4,
    )
    NW, TW, NT, NCH, TCW = cfg.nw, cfg.tw, cfg.ntiles, cfg.nch, cfg.t_cw
    NS, SEG = cfg.ns, cfg.seg
    S16 = SEG // 16

    ego0_d = nc.dram_tensor("ego0", [NS, D], f32, kind="ExternalInput")
    tidx_d = nc.dram_tensor(
        "tidx", [P, NW * NCH * S16], i16, kind="ExternalInput"
    )
    hmod_d = nc.dram_tensor("hmod", [P, NT], bf16, kind="ExternalInput")
    iota_d = nc.dram_tensor("iota", [P, P], bf16, kind="ExternalInput")
    ident_d = nc.dram_tensor("ident", [P, P], bf16, kind="ExternalInput")
    out_layers = [
        nc.dram_tensor(f"out{l}", [NS, D], f32, kind="ExternalOutput")
        for l in range(cfg.n_layers)
    ]
    if debug_dump:
        degdump_d = nc.dram_tensor("degdump", [P, NW * F], f32, kind="ExternalOutput")
        gdump_d = nc.dram_tensor("gdump", [P, TW, D], bf16, kind="ExternalOutput")
        sdump_d = nc.dram_tensor("sdump", [P, D], f32, kind="ExternalOutput")
    ybounce_d = nc.dram_tensor("ybounce", [NS, D], bf16)
    yfull_d = nc.dram_tensor("yfull", [cfg.n_pad, D], bf16)
    tau_d = nc.dram_tensor("taud", [NW, P, TW * D], bf16)

    # DRAM [NS, D] viewed as [P, NW, D]: row w*128+p, col d -> p, (w, d)
    def node_ap(dram):
        return dram.ap().rearrange("(w p) d -> p w d", p=P)

    rg = [list(range(cfg.cores))]
    n_it_total = cfg.n_layers * cfg.n_iters

    with tile.TileContext(nc) as tc:
        with (
            tc.tile_pool(name="pers", bufs=1) as pers,
            tc.tile_pool(name="work", bufs=2) as work,
            tc.tile_pool(name="wide", bufs=2) as wide,
            tc.tile_pool(name="gpool", bufs=6) as gpool,
            tc.tile_pool(name="ipool", bufs=4) as ipool,
            tc.tile_pool(name="taupool", bufs=3) as taupool,
            tc.tile_pool(name="small", bufs=4) as small,
            tc.tile_pool(name="psA", bufs=2, space="PSUM") as psA,
            tc.tile_pool(name="psS", bufs=2, space="PSUM") as psS,
        ):
            shat = pers.tile([P, NW, D], bf16, tag="shat")
            A = pers.tile([P, NT, F], bf16, tag="A")
            scores = pers.tile([P, NT, F], bf16, tag="scores")
            hmod = pers.tile([P, NT], bf16, tag="hmod")
            iota = pers.tile([P, P], bf16, tag="iota")
            ident = pers.tile([P, P], bf16, tag="ident")
            deg = pers.tile([P, NW, F], f32, tag="deg")
            dinv = pers.tile([P, NW, F], f32, tag="dinv")

            nc.sync.dma_start(hmod[:], hmod_d.ap())
            nc.sync.dma_start(iota[:], iota_d.ap())
            nc.sync.dma_start(ident[:], ident_d.ap())
            nc.vector.memset(A[:], 1.0)

            def build_M(t):
                M = work.tile([P, P], bf16, tag="M")
                nc.vector.tensor_tensor(
                    M[:], iota[:], hmod[:, t : t + 1].to_broadcast([P, P]),
                    mybir.AluOpType.is_equal,
                )
                return M

            def rep_ap(a, pos, count):
                # insert a step-0 dim at free position `pos` (0-based after partition)
                return bass.AP(
                    a.tensor, a.offset,
                    list(a.ap[: 1 + pos]) + [[0, count]] + list(a.ap[1 + pos :]),
                )

            def build_M_all(w):
                # [P, TW*128] one-hot blocks for all tiles of window w
                Mall = wide.tile([P, TW, P], bf16, tag="Mall")
                nc.vector.tensor_tensor(
                    Mall[:],
                    rep_ap(iota[:], 0, TW),
                    rep_ap(hmod[:, w * TW : (w + 1) * TW], 1, P),
                    mybir.AluOpType.is_equal,
                )
                return Mall

            for itg in range(n_it_total):
                it_in_layer = itg % cfg.n_iters
                layer = itg // cfg.n_iters
                last = itg == n_it_total - 1
                layer_end = it_in_layer == cfg.n_iters - 1

                # ---- scores = softmax(A) + deg pass (itg 0 only; later
                # iterations get scores/deg from the previous phase B) ----
                if itg == 0:
                    CH = 98
                    while NT % CH != 0:
                        CH -= 1
                    for q in range(NT // CH):
                        sl = slice(q * CH, (q + 1) * CH)
                        e32 = work.tile([P, CH, F], f32, tag="smx_e")
                        nc.scalar.activation(
                            e32[:], A[:, sl, :], mybir.ActivationFunctionType.Exp
                        )
                        ssum = work.tile([P, CH], f32, tag="smx_s")
                        nc.vector.tensor_reduce(
                            ssum[:], e32[:], mybir.AxisListType.X, mybir.AluOpType.add
                        )
                        rec = work.tile([P, CH], f32, tag="smx_r")
                        nc.vector.reciprocal(rec[:], ssum[:])
                        nc.vector.tensor_tensor(
                            scores[:, sl, :],
                            e32[:],
                            rec[:].to_broadcast([P, CH, F]),
                            mybir.AluOpType.mult,
                        )
                    if "deg" in ablate:
                        nc.vector.memset(deg[:], 1.0)
                    for w in range(NW) if "deg" not in ablate else []:
                        psd = psA.tile([P, F], f32, tag="psd")
                        Mall = build_M_all(w)
                        for k in range(TW):
                            t = w * TW + k
                            nc.tensor.matmul(
                                psd[:], Mall[:, k, :], scores[:, t, :],
                                start=(k == 0), stop=(k == TW - 1),
                            )
                        nc.scalar.activation(
                            deg[:, w, :], psd[:], mybir.ActivationFunctionType.Copy
                        )
                if debug_dump and itg == 0:
                    nc.sync.dma_start(
                        degdump_d.ap(), deg[:].rearrange("p w f -> p (w f)")
                    )

                # ---- per-group d_inv + y = ego * d_inv -> allgather.
                # dinv/y for group g depend only on deg[ws:ws+YB], so the
                # scheduler can overlap them with the tail of the previous
                # message loop instead of waiting for the last window's deg.
                ego_src = ego0_d if layer == 0 else out_layers[layer - 1]
                YB = 14 if NW % 14 == 0 else 1
                for wg in range(NW // YB):
                    ws = wg * YB
                    dtmp = work.tile([P, YB, F], f32, tag="dtmp")
                    nc.vector.tensor_scalar(
                        dtmp[:], deg[:, ws : ws + YB, :], EPS, None,
                        mybir.AluOpType.max,
                    )
                    nc.scalar.sqrt(dtmp[:], dtmp[:])
                    nc.vector.reciprocal(dinv[:, ws : ws + YB, :], dtmp[:])
                    egw = wide.tile([P, YB, D], f32, tag="wC")
                    nc.sync.dma_start(egw[:], node_ap(ego_src)[:, ws : ws + YB, :])
                    ysw = wide.tile([P, YB, D], bf16, tag="wB")
                    nc.vector.tensor_tensor(
                        ysw[:].rearrange("p w (f g) -> p w f g", f=F),
                        egw[:].rearrange("p w (f g) -> p w f g", f=F),
                        rep_ap(dinv[:, ws : ws + YB, :], 2, DF),
                        mybir.AluOpType.mult,
                    )
                    nc.sync.dma_start(node_ap(ybounce_d)[:, ws : ws + YB, :], ysw[:])
                if "allgather" in ablate:
                    nc.sync.dma_start(yfull_d.ap()[:NS, :], ybounce_d.ap())
                else:
                    nc.gpsimd.collective_compute(
                        "AllGather",
                        mybir.AluOpType.bypass,
                        replica_groups=rg,
                        ins=[ybounce_d.ap().opt()],
                        outs=[yfull_d.ap().opt()],
                    )

                # ---- message pass ----
                for w in range(NW):
                    idxs = ipool.tile([P, NCH * S16], i16, tag="idxs")
                    nc.sync.dma_start(
                        idxs[:],
                        tidx_d.ap()[:, w * NCH * S16 : (w + 1) * NCH * S16],
                    )
                    G = gpool.tile([P, TW, D], bf16, tag="G")
                    for c in range(NCH) if "gather" not in ablate else []:
                        nc.gpsimd.dma_gather(
                            out_ap=G[:, c * TCW : (c + 1) * TCW, :],
                            in_ap=yfull_d.ap()[
                                c * cfg.chunk : (c + 1) * cfg.chunk, :
                            ],
                            idxs_ap=idxs[:, c * S16 : (c + 1) * S16],
                            num_idxs=SEG,
                            num_idxs_reg=SEG,
                            elem_size=D,
                            queue_num=c,
                        )
                    if debug_dump and itg == 0 and w == 0:
                        nc.sync.dma_start(gdump_d.ap(), G[:])
                    ps = psS.tile([P, D], f32, tag="psS")
                    if "msgzero" in ablate:
                        nc.vector.memset(ps[:], 0.0)
                    if it_in_layer == 0 and "msgcompute" not in ablate:
                        taub = taupool.tile([P, TW * D], bf16, tag="taub")
                    if "msgcompute" not in ablate:
                        Mall = build_M_all(w)
                        rhs = wide.tile([P, TW, D], bf16, tag="wB")
                        nc.vector.tensor_tensor(
                            rhs[:].rearrange("p t (f g) -> p t f g", f=F),
                            G[:].rearrange("p t (f g) -> p t f g", f=F),
                            rep_ap(scores[:, w * TW : (w + 1) * TW, :], 2, DF),
                            mybir.AluOpType.mult,
                        )
                        if it_in_layer == 0:
                            sq_all = wide.tile([P, TW, D], bf16, tag="wA")
                            nc.scalar.activation(
                                sq_all[:], G[:],
                                mybir.ActivationFunctionType.Square,
                            )
                            sqf = work.tile([P, TW, F, DF // 2], bf16, tag="fold16")
                            sqv = sq_all[:].rearrange(
                                "p t (f g h) -> p t f g h", f=F, g=2
                            )
                            nc.vector.tensor_tensor(
                                sqf[:],
                                sqv[:, :, :, 0, :],
                                sqv[:, :, :, 1, :],
                                mybir.AluOpType.add,
                            )
                            n2 = work.tile([P, TW, F], f32, tag="n2w")
                            nc.vector.tensor_reduce(
                                n2[:], sqf[:], mybir.AxisListType.X,
                                mybir.AluOpType.add,
                            )
                            nc.vector.tensor_scalar(
                                n2[:], n2[:], EPS * EPS, None, mybir.AluOpType.max
                            )
                            nc.scalar.sqrt(n2[:], n2[:])
                            rn = work.tile([P, TW, F], f32, tag="rnw")
                            nc.vector.reciprocal(rn[:], n2[:])
                            that = wide.tile([P, TW, D], bf16, tag="wC")
                            nc.vector.tensor_tensor(
                                that[:].rearrange("p t (f g) -> p t f g", f=F),
                                G[:].rearrange("p t (f g) -> p t f g", f=F),
                                rep_ap(rn[:], 2, DF),
                                mybir.AluOpType.mult,
                            )
                            nc.scalar.activation(
                                taub[:], that[:].rearrange("p t d -> p (t d)"),
                                mybir.ActivationFunctionType.Tanh,
                            )
                        for k in range(TW):
                            nc.tensor.matmul(
                                ps[:], Mall[:, k, :], rhs[:, k, :],
                                start=(k == 0), stop=(k == TW - 1),
                            )
                    # spill tau only when a later non-final iteration of this
                    # layer will reload it (layer0 it1); fused phase B below
                    # consumes the SBUF copy directly on it0/it2
                    if (
                        it_in_layer == 0
                        and layer < cfg.n_layers - 1
                        and "msgcompute" not in ablate
                    ):
                        nc.sync.dma_start(tau_d.ap()[w], taub[:])
                    # node phase for this window
                    if debug_dump and itg == 0 and w == 0:
                        sdmp = work.tile([P, D], f32, tag="sdmp")
                        nc.scalar.activation(
                            sdmp[:], ps[:], mybir.ActivationFunctionType.Copy
                        )
                        nc.sync.dma_start(sdump_d.ap(), sdmp[:])
                    n2s = small.tile([P, F], f32, tag="n2s")
                    sqs = work.tile([P, D], f32, tag="sqs")
                    nc.scalar.activation(
                        sqs[:], ps[:], mybir.ActivationFunctionType.Square
                    )
                    nc.vector.tensor_reduce(
                        n2s[:],
                        sqs[:].rearrange("p (f g) -> p f g", f=F),
                        mybir.AxisListType.X,
                        mybir.AluOpType.add,
                    )
                    rns = small.tile([P, F], f32, tag="rns")
                    nc.vector.tensor_scalar(
                        n2s[:], n2s[:], EPS * EPS, None, mybir.AluOpType.max
                    )
                    nc.scalar.sqrt(n2s[:], n2s[:])
                    nc.vector.reciprocal(rns[:], n2s[:])
                    nc.vector.tensor_tensor(
                        shat[:, w, :].rearrange("p (f g) -> p f g", f=F),
                        ps[:].rearrange("p (f g) -> p f g", f=F),
                        rns[:].to_broadcast([P, F, DF]),
                        mybir.AluOpType.mult,
                    )
                    if layer_end:
                        xnw = work.tile([P, D], f32, tag="xnw")
                        nc.vector.tensor_tensor(
                            xnw[:].rearrange("p (f g) -> p f g", f=F),
                            ps[:].rearrange("p (f g) -> p f g", f=F),
                            dinv[:, w, :].to_broadcast([P, F, DF]),
                            mybir.AluOpType.mult,
                        )
                        nc.sync.dma_start(
                            node_ap(out_layers[layer])[:, w, :], xnw[:]
                        )

                    # ---- fused phase B: A += chunk_dot(M @ shat_w, tau_w),
                    # then next-iteration scores/deg for this window.
                    # Mt built on PE (transpose vs identity); tau comes from
                    # SBUF on it0 of a layer, from the it0 spill otherwise.
                    if last or "phaseB" in ablate or "msgcompute" in ablate:
                        continue
                    if it_in_layer == 0:
                        taur = taub[:].rearrange("p (t d) -> p t d", d=D)
                    else:
                        taub2 = taupool.tile([P, TW * D], bf16, tag="taub")
                        nc.sync.dma_start(taub2[:], tau_d.ap()[w])
                        taur = taub2[:].rearrange("p (t d) -> p t d", d=D)
                    headb = wide.tile([P, TW, D], bf16, tag="wD")
                    for kb in range(0, TW, 4):
                        psT = psS.tile([P, 4, P], bf16, tag="psT")
                        for j in range(4):
                            nc.tensor.transpose(
                                psT[:, j, :], Mall[:, kb + j, :], ident[:]
                            )
                        MtB = work.tile([P, 4, P], bf16, tag="Mt")
                        nc.vector.tensor_copy(MtB[:], psT[:])
                        psb = psS.tile([P, 4, D], f32, tag="psb")
                        for j in range(4):
                            nc.tensor.matmul(
                                psb[:, j, :], MtB[:, j, :], shat[:, w, :],
                                start=True, stop=True,
                            )
                        nc.vector.tensor_copy(headb[:, kb : kb + 4, :], psb[:])
                    prod = wide.tile([P, TW, D], bf16, tag="wA")
                    nc.vector.tensor_tensor(
                        prod[:], headb[:], taur, mybir.AluOpType.mult
                    )
                    prf = work.tile([P, TW, F, DF // 2], bf16, tag="fold16")
                    prv = prod[:].rearrange("p t (f g h) -> p t f g h", f=F, g=2)
                    nc.vector.tensor_tensor(
                        prf[:],
                        prv[:, :, :, 0, :],
                        prv[:, :, :, 1, :],
                        mybir.AluOpType.add,
                    )
                    nv = work.tile([P, TW, F], f32, tag="n2w")
                    nc.vector.tensor_reduce(
                        nv[:], prf[:], mybir.AxisListType.X, mybir.AluOpType.add
                    )
                    nc.vector.tensor_tensor(
                        A[:, w * TW : (w + 1) * TW, :],
                        A[:, w * TW : (w + 1) * TW, :],
                        nv[:],
                        mybir.AluOpType.add,
                    )
                    # fused: next iteration's scores + deg for this window
                    sl = slice(w * TW, (w + 1) * TW)
                    e32 = work.tile([P, TW, F], f32, tag="smx_e")
                    nc.scalar.activation(
                        e32[:], A[:, sl, :], mybir.ActivationFunctionType.Exp
                    )
                    ssum = work.tile([P, TW], f32, tag="smx_s")
                    nc.vector.tensor_reduce(
                        ssum[:], e32[:], mybir.AxisListType.X,
                        mybir.AluOpType.add,
                    )
                    rec = work.tile([P, TW], f32, tag="smx_r")
                    nc.vector.reciprocal(rec[:], ssum[:])
                    nc.vector.tensor_tensor(
                        scores[:, sl, :],
                        e32[:],
                        rec[:].to_broadcast([P, TW, F]),
                        mybir.AluOpType.mult,
                    )
                    psd = psA.tile([P, F], f32, tag="psd")
                    for k in range(TW):
                        nc.tensor.matmul(
                            psd[:], Mall[:, k, :], scores[:, w * TW + k, :],
                            start=(k == 0), stop=(k == TW - 1),
                        )
                    nc.scalar.activation(
                        deg[:, w, :], psd[:], mybir.ActivationFunctionType.Copy
                    )

    nc.compile()
    return nc


# ---------------------------------------------------------------- runner


def make_in_maps(user_emb, item_emb, all_h_list, all_t_list, cfg: Cfg):
    edges = preprocess(all_h_list, all_t_list, cfg)
    egos = shard_ego(user_emb, item_emb, cfg)
    iota = np.broadcast_to(np.arange(P, dtype=np.float32), (P, P)).astype(
        ml_dtypes.bfloat16
    ).copy()
    in_maps = []
    ident = np.eye(P, dtype=np.float32).astype(ml_dtypes.bfloat16)
    for c in range(cfg.cores):
        in_maps.append(
            {
                "ego0": egos[c],
                "tidx": edges[c]["tidx"],
                "hmod": edges[c]["hmod"],
                "iota": iota.copy(),
                "ident": ident.copy(),
            }
        )
    return in_maps, egos


def assemble_output(results, egos, cfg: Cfg):
    n_layers = cfg.n_layers
    acc = []
    for c in range(cfg.cores):
        s = egos[c].astype(np.float64).copy()
        for l in range(n_layers):
            s += np.asarray(results[c][f"out{l}"], dtype=np.float64)
        acc.append(s / (n_layers + 1))
    full = np.concatenate(acc, 0)[:N_REAL].astype(np.float32)
    u_g = full[:N_USERS]
    i_g = full[N_USERS:]
    return u_g, i_g, u_g, i_g


_CACHE = {}


def kernel(user_emb, item_emb, all_h_list, all_t_list):
    h = np.asarray(all_h_list)
    t = np.asarray(all_t_list)
    cfg0 = Cfg(8, 98, 1)
    t_cw = compute_t_cw(h, t, cfg0)
    cfg = Cfg(8, 98, t_cw)

    key = (cfg.cores, cfg.nw, cfg.t_cw)
    if key not in _CACHE:
        _CACHE[key] = build_program(cfg)
    nc = _CACHE[key]

    in_maps, egos = make_in_maps(user_emb, item_emb, h, t, cfg)
    last_err = None
    for _ in range(3):
        try:
            res = run_bass_kernel_spmd(nc, in_maps, list(range(cfg.cores)))
            return assemble_output(res.results, egos, cfg)
        except Exception as e:  # transient NRT device errors observed
            last_err = e
    raise last_err

